# revision 4
# baseline (speedup 1.0000x reference)
"""CBFGraphNet Trainium2 kernel.

Math notes (exact rewrites of the reference, no approximation beyond fp
reassociation):

  The reference returns a scalar computed from nodes[0] only ("drone").
  Edge states are never updated from node states, so the final value
  depends only on:
    - node_feats[0]
    - S0 = sum of edge_feats rows whose receiver == 0
    - c0 = number of edges whose receiver == 0
    - the (tiny) weight matrices
  via segment_sum linearity:
    segment_sum(edge_feats @ W + b)[0] == S0 @ W + c0 * b

  The O(E) work — streaming all edge_feats (205 MB) + receivers (13 MB)
  and computing the masked sum S0 / count c0 — runs on 8 NeuronCores,
  edges sharded evenly.  Each core emits 128 partition-partials of
  [S0 | c0]; the host adds 8*128 partials and finishes the O(1) MLP.

Per-core device program (SPMD, identical on all 8 cores):
  - edge slice laid out [128 partitions, 3125 edges/partition, 16 feats]
  - 5 chunks of 625 edge-columns: DMA in (~5 MB/chunk), then on DVE:
      mask = (recv == 0) as f32
      for f in 0..15: acc[:, f] += sum_j mask[:, j] * x[:, j, f]
      acc[:, 16]     += sum_j mask[:, j]
    using fused tensor_tensor_reduce with running accumulation.
"""

import sys

if "/opt/trn_rl_repo" not in sys.path:
    sys.path.insert(0, "/opt/trn_rl_repo")

import numpy as np

N_NODES = 100_000
N_EDGES = 3_200_000
F_IN = 16
HID = 64
N_CORES = 8
P = 128

EC = N_EDGES // N_CORES          # 400_000 edges per core
JPC = EC // P                    # 3125 edges per partition
M = 625                          # edges per partition per chunk
NCHUNK = JPC // M                # 5

_CACHE: dict = {}
LAST_RESULTS = None              # BassKernelResults from the latest run


def _build_bass():
    import concourse.bacc as bacc
    import concourse.mybir as mybir
    from concourse.tile import TileContext

    f32 = mybir.dt.float32
    i32 = mybir.dt.int32

    nc = bacc.Bacc("TRN2", target_bir_lowering=False)
    ef = nc.declare_dram_parameter("ef", [P, JPC * F_IN], f32, isOutput=False)
    rv = nc.declare_dram_parameter("rv", [P, JPC], i32, isOutput=False)
    out = nc.declare_dram_parameter("out", [P, F_IN + 1], f32, isOutput=True)

    with TileContext(nc) as tc:
        with tc.tile_pool(name="x", bufs=2) as xp, \
             tc.tile_pool(name="small", bufs=2) as sp, \
             tc.tile_pool(name="persist", bufs=1) as pp:
            acc = pp.tile([P, F_IN + 1], f32)
            nc.vector.memset(acc[:], 0.0)
            for c in range(NCHUNK):
                x = xp.tile([P, M * F_IN], f32)
                r = sp.tile([P, M], i32, tag="recv")
                mk = sp.tile([P, M], f32, tag="mask")
                red = sp.tile([P, F_IN + 1], f32, tag="red")
                nc.sync.dma_start(
                    out=x[:], in_=ef[:, c * M * F_IN:(c + 1) * M * F_IN])
                nc.sync.dma_start(out=r[:], in_=rv[:, c * M:(c + 1) * M])
                nc.vector.tensor_scalar(
                    out=mk[:], in0=r[:], scalar1=0, scalar2=None,
                    op0=mybir.AluOpType.is_equal)
                x3 = x[:].rearrange("p (j f) -> p j f", f=F_IN)
                nc.vector.tensor_tensor(
                    out=x3, in0=x3, in1=mk[:].broadcast_to((P, M, F_IN)),
                    op=mybir.AluOpType.mult)
                nc.vector.tensor_reduce(
                    out=red[:, 0:F_IN],
                    in_=x[:].rearrange("p (j f) -> p f j", f=F_IN),
                    axis=mybir.AxisListType.X, op=mybir.AluOpType.add)
                nc.vector.tensor_reduce(
                    out=red[:, F_IN:F_IN + 1], in_=mk[:],
                    axis=mybir.AxisListType.X, op=mybir.AluOpType.add)
                nc.vector.tensor_tensor(
                    out=acc[:], in0=acc[:], in1=red[:],
                    op=mybir.AluOpType.add)
            nc.sync.dma_start(out=out[:], in_=acc[:])
    nc.compile()
    return nc


def _get_bass():
    if "nc" not in _CACHE:
        _CACHE["nc"] = _build_bass()
    return _CACHE["nc"]


def kernel(node_feats, edge_feats, receivers,
           node_W, node_b, edge_W, edge_b,
           msg_W0, msg_b0, msg_W1, msg_b1,
           upd_W0, upd_b0, upd_W1, upd_b1,
           cbf_W1, cbf_b1, cbf_W2, cbf_b2,
           _trace=False, _trace_cores=None):
    global LAST_RESULTS
    from concourse.bass_utils import run_bass_kernel_spmd

    edge_feats = np.ascontiguousarray(edge_feats, dtype=np.float32)
    receivers = np.ascontiguousarray(receivers, dtype=np.int32)

    ef_sh = edge_feats.reshape(N_CORES, P, JPC * F_IN)
    rv_sh = receivers.reshape(N_CORES, P, JPC)
    in_maps = [{"ef": ef_sh[k], "rv": rv_sh[k]} for k in range(N_CORES)]

    nc = _get_bass()
    res = run_bass_kernel_spmd(
        nc, in_maps, list(range(N_CORES)),
        trace=_trace, trace_cores=_trace_cores)
    LAST_RESULTS = res

    partials = np.stack([np.asarray(r["out"]) for r in res.results])  # [8,128,17]
    partials = partials.sum(axis=(0, 1), dtype=np.float64)
    S0 = partials[:F_IN].astype(np.float32)
    c0 = np.float32(partials[F_IN])

    # O(1) finish: node-0 slice of the reference network.
    e_enc = S0 @ edge_W + c0 * edge_b
    n0 = node_feats[0] @ node_W + node_b
    for mW, mb, uW, ub in ((msg_W0, msg_b0, upd_W0, upd_b0),
                           (msg_W1, msg_b1, upd_W1, upd_b1)):
        agg = e_enc @ mW + c0 * mb
        n0 = np.maximum((n0 + agg) @ uW + ub, np.float32(0.0))
    h = np.maximum(n0 @ cbf_W1 + cbf_b1, np.float32(0.0))
    val = h @ cbf_W2 + cbf_b2
    return np.float32(val[0])


# revision 5
# speedup vs baseline: 6.5291x; 6.5291x over previous
"""CBFGraphNet Trainium2 kernel.

Math notes (exact rewrites of the reference, no approximation beyond fp
reassociation):

  The reference returns a scalar computed from nodes[0] only ("drone").
  Edge states are never updated from node states, so the final value
  depends only on:
    - node_feats[0]
    - S0 = sum of edge_feats rows whose receiver == 0
    - c0 = number of edges whose receiver == 0
    - the (tiny) weight matrices
  via segment_sum linearity:
    segment_sum(edge_feats @ W + b)[0] == S0 @ W + c0 * b

Device work (8 NeuronCores, edges sharded evenly, SPMD):

  Primary path ("compaction"): each core scans its receivers slice
  [128 partitions x 3125] on the vector engine — mask = (recv == 0),
  per-partition count, and the top-8 match positions via max/max_index.
  The host turns (count, indices) into global edge ids, gathers those
  few edge_feats rows (O(#matches) work), and finishes the O(1) MLP.

  Fallback path ("streaming", used only if some partition row has more
  than 8 matches so the 8-slot index list would be incomplete): stream
  all edge_feats too and compute S0 as a masked sum on-device.
"""

import sys

if "/opt/trn_rl_repo" not in sys.path:
    sys.path.insert(0, "/opt/trn_rl_repo")

import numpy as np

N_NODES = 100_000
N_EDGES = 3_200_000
F_IN = 16
HID = 64
N_CORES = 8
P = 128

EC = N_EDGES // N_CORES          # 400_000 edges per core
JPC = EC // P                    # 3125 edges per partition
M = 625                          # streaming path: edges/partition/chunk
NCHUNK = JPC // M                # 5

_CACHE: dict = {}
LAST_RESULTS = None              # BassKernelResults from the latest run


def _build_compact():
    import concourse.bacc as bacc
    import concourse.mybir as mybir
    from concourse.tile import TileContext

    f32 = mybir.dt.float32
    i32 = mybir.dt.int32
    u32 = mybir.dt.uint32

    nc = bacc.Bacc("TRN2", target_bir_lowering=False)
    rv = nc.declare_dram_parameter("rv", [P, JPC], i32, isOutput=False)
    ocnt = nc.declare_dram_parameter("ocnt", [P, 9], f32, isOutput=True)
    oidx = nc.declare_dram_parameter("oidx", [P, 8], u32, isOutput=True)
    with TileContext(nc) as tc:
        with tc.tile_pool(name="p", bufs=1) as pp:
            rt = pp.tile([P, JPC], i32)
            mk = pp.tile([P, JPC], f32)
            cv = pp.tile([P, 9], f32)
            ix = pp.tile([P, 8], u32)
            nc.sync.dma_start(out=rt[:], in_=rv[:])
            nc.vector.tensor_scalar(out=mk[:], in0=rt[:], scalar1=0,
                                    scalar2=None,
                                    op0=mybir.AluOpType.is_equal)
            nc.vector.tensor_reduce(out=cv[:, 0:1], in_=mk[:],
                                    axis=mybir.AxisListType.X,
                                    op=mybir.AluOpType.add)
            nc.vector.max(cv[:, 1:9], mk[:])
            nc.vector.max_index(ix[:], cv[:, 1:9], mk[:])
            nc.sync.dma_start(out=ocnt[:], in_=cv[:])
            nc.sync.dma_start(out=oidx[:], in_=ix[:])
    nc.compile()
    return nc


def _build_stream():
    import concourse.bacc as bacc
    import concourse.mybir as mybir
    from concourse.tile import TileContext

    f32 = mybir.dt.float32
    i32 = mybir.dt.int32

    nc = bacc.Bacc("TRN2", target_bir_lowering=False)
    ef = nc.declare_dram_parameter("ef", [P, JPC * F_IN], f32, isOutput=False)
    rv = nc.declare_dram_parameter("rv", [P, JPC], i32, isOutput=False)
    out = nc.declare_dram_parameter("out", [P, F_IN + 1], f32, isOutput=True)

    with TileContext(nc) as tc:
        with tc.tile_pool(name="x", bufs=2) as xp, \
             tc.tile_pool(name="small", bufs=2) as sp, \
             tc.tile_pool(name="persist", bufs=1) as pp:
            acc = pp.tile([P, F_IN + 1], f32)
            nc.vector.memset(acc[:], 0.0)
            for c in range(NCHUNK):
                x = xp.tile([P, M * F_IN], f32)
                r = sp.tile([P, M], i32, tag="recv")
                mk = sp.tile([P, M], f32, tag="mask")
                red = sp.tile([P, F_IN + 1], f32, tag="red")
                nc.sync.dma_start(
                    out=x[:], in_=ef[:, c * M * F_IN:(c + 1) * M * F_IN])
                nc.sync.dma_start(out=r[:], in_=rv[:, c * M:(c + 1) * M])
                nc.vector.tensor_scalar(
                    out=mk[:], in0=r[:], scalar1=0, scalar2=None,
                    op0=mybir.AluOpType.is_equal)
                x3 = x[:].rearrange("p (j f) -> p j f", f=F_IN)
                nc.vector.tensor_tensor(
                    out=x3, in0=x3, in1=mk[:].broadcast_to((P, M, F_IN)),
                    op=mybir.AluOpType.mult)
                nc.vector.tensor_reduce(
                    out=red[:, 0:F_IN],
                    in_=x[:].rearrange("p (j f) -> p f j", f=F_IN),
                    axis=mybir.AxisListType.X, op=mybir.AluOpType.add)
                nc.vector.tensor_reduce(
                    out=red[:, F_IN:F_IN + 1], in_=mk[:],
                    axis=mybir.AxisListType.X, op=mybir.AluOpType.add)
                nc.vector.tensor_tensor(
                    out=acc[:], in0=acc[:], in1=red[:],
                    op=mybir.AluOpType.add)
            nc.sync.dma_start(out=out[:], in_=acc[:])
    nc.compile()
    return nc


def _get(name, builder):
    if name not in _CACHE:
        _CACHE[name] = builder()
    return _CACHE[name]


def _finish(S0, c0, node_feats, node_W, node_b, edge_W, edge_b,
            msg_W0, msg_b0, msg_W1, msg_b1,
            upd_W0, upd_b0, upd_W1, upd_b1,
            cbf_W1, cbf_b1, cbf_W2, cbf_b2):
    # O(1) finish: node-0 slice of the reference network.
    e_enc = S0 @ edge_W + c0 * edge_b
    n0 = node_feats[0] @ node_W + node_b
    for mW, mb, uW, ub in ((msg_W0, msg_b0, upd_W0, upd_b0),
                           (msg_W1, msg_b1, upd_W1, upd_b1)):
        agg = e_enc @ mW + c0 * mb
        n0 = np.maximum((n0 + agg) @ uW + ub, np.float32(0.0))
    h = np.maximum(n0 @ cbf_W1 + cbf_b1, np.float32(0.0))
    val = h @ cbf_W2 + cbf_b2
    return np.float32(val[0])


def kernel(node_feats, edge_feats, receivers,
           node_W, node_b, edge_W, edge_b,
           msg_W0, msg_b0, msg_W1, msg_b1,
           upd_W0, upd_b0, upd_W1, upd_b1,
           cbf_W1, cbf_b1, cbf_W2, cbf_b2,
           _trace=False, _trace_cores=None, _force_stream=False):
    global LAST_RESULTS
    from concourse.bass_utils import run_bass_kernel_spmd

    edge_feats = np.ascontiguousarray(edge_feats, dtype=np.float32)
    receivers = np.ascontiguousarray(receivers, dtype=np.int32)
    rv_sh = receivers.reshape(N_CORES, P, JPC)

    weights = dict(
        node_feats=node_feats, node_W=node_W, node_b=node_b,
        edge_W=edge_W, edge_b=edge_b,
        msg_W0=msg_W0, msg_b0=msg_b0, msg_W1=msg_W1, msg_b1=msg_b1,
        upd_W0=upd_W0, upd_b0=upd_b0, upd_W1=upd_W1, upd_b1=upd_b1,
        cbf_W1=cbf_W1, cbf_b1=cbf_b1, cbf_W2=cbf_W2, cbf_b2=cbf_b2)

    if not _force_stream:
        nc = _get("compact", _build_compact)
        in_maps = [{"rv": rv_sh[k]} for k in range(N_CORES)]
        res = run_bass_kernel_spmd(
            nc, in_maps, list(range(N_CORES)),
            trace=_trace, trace_cores=_trace_cores)
        LAST_RESULTS = res
        cnts = np.stack([np.asarray(r["ocnt"]) for r in res.results])  # [8,P,9]
        idxs = np.stack([np.asarray(r["oidx"]) for r in res.results])  # [8,P,8]
        counts = np.rint(cnts[:, :, 0]).astype(np.int64)               # [8,P]
        if counts.max() <= 8:
            S0 = np.zeros(F_IN, np.float32)
            c0 = np.float32(counts.sum())
            ks, ps = np.nonzero(counts)
            for k, p in zip(ks, ps):
                c = counts[k, p]
                js = idxs[k, p, :c].astype(np.int64)
                e = (k * P + p) * JPC + js
                S0 += edge_feats[e].sum(axis=0, dtype=np.float32)
            return _finish(S0, c0, **weights)
        # else: >8 matches in one partition row — index list incomplete,
        # fall through to the streaming path.

    nc = _get("stream", _build_stream)
    ef_sh = edge_feats.reshape(N_CORES, P, JPC * F_IN)
    in_maps = [{"ef": ef_sh[k], "rv": rv_sh[k]} for k in range(N_CORES)]
    res = run_bass_kernel_spmd(
        nc, in_maps, list(range(N_CORES)),
        trace=_trace, trace_cores=_trace_cores)
    LAST_RESULTS = res
    partials = np.stack([np.asarray(r["out"]) for r in res.results])
    partials = partials.sum(axis=(0, 1), dtype=np.float64)
    S0 = partials[:F_IN].astype(np.float32)
    c0 = np.float32(partials[F_IN])
    return _finish(S0, c0, **weights)
